# revision 7
# baseline (speedup 1.0000x reference)
"""Trainium2 Bass kernel for nn_ExtractionLayer.

metric[b,v,f] = sum_p amp[b,f,p] * exp(-c*(vol[v]*filt[f] - q[b,p])^2)
  amp = softmax_p(logits[b,f,p]),  c = 0.5/(sigma+0.001)^2

Sharding: data-parallel over batch B=32 -> 4 b's per core on 8 cores.

Per-core algorithm (2 "sets", each set = 2 b's = 128 (b,p) partition pairs):
  PE pass 1 : S[(b,p),(f,v)] = x^2 - 2qx + q^2 via K=3 matmul
              (lhsT rows {1,-2q,q^2} per (b,p) col, rhs rows {x^2,x,1})
  ACT pass  : E = exp(-c*S)  PSUM->SBUF (fp16), the dominant exp pass
  PE pass 2 : per (f, v-half): lhsT = E-slice (128,128) stationary,
              rhs = block-diag softmax weight pair (128,2) moving ->
              psum out (128 v, 2 b) -- partition-dense output
  drain     : DVE copy psum->SBUF, DMA -> out[v,f,b]; host -> [b,v,f]
"""

import sys

for _p in ("/opt/trn_rl_repo", "/root/.axon_site/_ro/trn_rl_repo"):
    if _p not in sys.path:
        sys.path.append(_p)

import numpy as np

B, V, F, P = 32, 256, 128, 64
NCORES = 8
B_LOC = B // NCORES          # 4 batches per core
NSETS = B_LOC // 2           # 2 sets of (2 b's x 64 p) = 128 partitions
NVF = V * F                  # 32768 (f-major: i = f*V + v)
CHUNK = 512                  # PE-1 matmul free dim (= 2 f's)
GROUP = 2048                 # ACT free dim (= 8 f's, 4 chunks, 4 PSUM banks)
NGROUPS = NVF // GROUP       # 16 per set
F_PER_GROUP = GROUP // V     # 8
F_PER_OTILE = 4              # psO tile covers 4 f's -> (2, 1024) = 2 banks

_cache: dict = {}


def _build():
    import concourse.tile as tile
    from concourse import bacc, mybir

    fp32 = mybir.dt.float32
    fp16 = mybir.dt.float16
    AF = mybir.ActivationFunctionType
    OP = mybir.AluOpType
    import concourse.bass as bass

    nc = bacc.Bacc("TRN2", target_bir_lowering=False, debug=False,
                   num_devices=NCORES)

    d_q = nc.dram_tensor("q", [B_LOC * P], fp32, kind="ExternalInput")
    d_lf = nc.dram_tensor("lf", [F, B_LOC, P], fp32, kind="ExternalInput")
    d_lt = nc.dram_tensor("lt", [B_LOC * P, F], fp32, kind="ExternalInput")
    d_vol = nc.dram_tensor("vol", [V], fp32, kind="ExternalInput")
    d_fil = nc.dram_tensor("fil", [F], fp32, kind="ExternalInput")
    d_sig = nc.dram_tensor("sig", [1], fp32, kind="ExternalInput")
    d_out = nc.dram_tensor("out", [V, F, B_LOC], fp32, kind="ExternalOutput")
    d_zb = nc.dram_tensor("zb", [F * B_LOC], fp32)  # Zinv bounce buffer

    with tile.TileContext(nc) as tc:
        with (
            tc.tile_pool(name="const", bufs=1) as cp,
            tc.tile_pool(name="ering", bufs=3) as ep,
            tc.tile_pool(name="psS", bufs=1, space=bass.MemorySpace.PSUM) as psS,
            tc.tile_pool(name="psO", bufs=2, space=bass.MemorySpace.PSUM) as psO,
        ):
            # ---- scalar constants from sigma, on all 128 partitions ----
            sigc = cp.tile([128, 1], fp32, tag="sigc")
            nc.sync.dma_start(
                sigc[:, :],
                bass.AP(tensor=d_sig, offset=0, ap=[[0, 128], [1, 1]]),
            )
            invc = cp.tile([128, 1], fp32, tag="invc")
            nc.vector.tensor_scalar_add(invc[:, :], sigc[:, :], 0.001)
            nc.vector.reciprocal(invc[:, :], invc[:, :])  # 1/(sig+.001)
            mc = cp.tile([128, 1], fp32, tag="mc")        # -c = -0.5*inv^2
            nc.vector.tensor_tensor(mc[:, :], invc[:, :], invc[:, :], OP.mult)
            nc.vector.tensor_scalar_mul(mc[:, :], mc[:, :], -0.5)

            # ---- X = rows {x^2, x, 1}, x[i=f*V+v] = fil[f]*vol[v] ----
            filc = cp.tile([128, 1], fp32, tag="filc")
            nc.sync.dma_start(filc[:, :], d_fil.ap().rearrange("(f o) -> f o", o=1))
            volr = cp.tile([128, V], fp32, tag="volr")
            nc.sync.dma_start(
                volr[:, :], bass.AP(tensor=d_vol, offset=0, ap=[[0, 128], [1, V]])
            )
            x_ft = cp.tile([128, V], fp32, tag="x_ft")
            nc.vector.tensor_scalar(x_ft[:, :], volr[:, :], filc[:, 0:1], None,
                                    op0=OP.mult)
            xsq_ft = cp.tile([128, V], fp32, tag="xsq_ft")
            nc.vector.tensor_tensor(xsq_ft[:, :], x_ft[:, :], x_ft[:, :], OP.mult)
            ones_ft = cp.tile([128, V], fp32, tag="ones_ft")
            nc.vector.memset(ones_ft[:, :], 1.0)

            X = cp.tile([3, NVF], fp32, tag="X")
            nc.sync.dma_start(X[0:1, :], xsq_ft[:, :])
            nc.sync.dma_start(X[1:2, :], x_ft[:, :])
            nc.sync.dma_start(X[2:3, :], ones_ft[:, :])

            # ---- W_q per set: rows {1, -2q, q^2} ----
            q_row = cp.tile([1, B_LOC * P], fp32, tag="q_row")
            nc.sync.dma_start(q_row[:, :], d_q.ap())
            Wq = []
            qtmp = cp.tile([1, 2 * 128], fp32, tag="qtmp")
            for s in range(NSETS):
                w = cp.tile([3, 128], fp32, tag=f"Wq{s}", name=f"Wq{s}")
                qs = q_row[0:1, s * 128:(s + 1) * 128]
                nc.vector.memset(w[0:1, :], 1.0)
                # rows built at partition 0, DMA'd into partitions 1/2
                # (engine ops may only start at partition 0/32/64/96)
                nc.vector.tensor_scalar_mul(qtmp[0:1, 0:128], qs, -2.0)
                nc.vector.tensor_tensor(qtmp[0:1, 128:256], qs, qs, OP.mult)
                nc.sync.dma_start(w[1:2, :], qtmp[0:1, 0:128])
                nc.sync.dma_start(w[2:3, :], qtmp[0:1, 128:256])
                Wq.append(w)

            # ---- softmax denominators: Z[f, b] = sum_p exp(logits) ----
            lf_sb = cp.tile([128, B_LOC, P], fp32, tag="lf_sb")
            nc.sync.dma_start(lf_sb[:, :, :], d_lf.ap())
            el_f = cp.tile([128, B_LOC, P], fp32, tag="el_f")
            nc.scalar.activation(el_f[:, :, :], lf_sb[:, :, :], AF.Exp)
            Z = cp.tile([128, B_LOC], fp32, tag="Z")
            nc.vector.tensor_reduce(Z[:, :], el_f[:, :, :], mybir.AxisListType.X,
                                    OP.add)
            Zinv = cp.tile([128, B_LOC], fp32, tag="Zinv")
            nc.vector.reciprocal(Zinv[:, :], Z[:, :])
            nc.sync.dma_start(d_zb.ap(), Zinv[:, :])  # bounce [f*B_LOC + b]

            # ---- W_amp per set: block-diag fp16 softmax weights ----
            # W_amp[k=(b,p), 2f+h] = amp[b,f,p] for k//64==h else 0
            Wamp = []
            for s in range(NSETS):
                lt_sb = cp.tile([128, F], fp32, tag=f"lt{s}")
                nc.sync.dma_start(lt_sb[:, :], d_lt.ap()[s * 128:(s + 1) * 128, :])
                elt = cp.tile([128, F], fp32, tag=f"elt{s}")
                nc.scalar.activation(elt[:, :], lt_sb[:, :], AF.Exp)
                zr = cp.tile([128, F], fp32, tag=f"zr{s}")
                for h in range(2):
                    nc.sync.dma_start(
                        zr[h * 64:(h + 1) * 64, :],
                        bass.AP(tensor=d_zb, offset=(2 * s + h),
                                ap=[[0, 64], [B_LOC, F]]),
                    )
                w = cp.tile([128, 2 * F], fp16, tag=f"Wamp{s}")
                nc.vector.memset(w[:, :], 0.0)
                for h in range(2):
                    nc.vector.tensor_tensor(
                        w[h * 64:(h + 1) * 64, h:2 * F:2],
                        elt[h * 64:(h + 1) * 64, :],
                        zr[h * 64:(h + 1) * 64, :],
                        OP.mult,
                    )
                Wamp.append(w)

            # ---- main pipeline ----
            for s in range(NSETS):
                # psum out accumulators: [v(128), f(128)*b'(2)] per v-half
                sO = [psO.tile([128, 2 * F], fp32, tag=f"O{vh}", name=f"sO{vh}")
                      for vh in range(2)]
                for g in range(NGROUPS):
                    sS = psS.tile([128, GROUP], fp32, tag="S")
                    for ci in range(GROUP // CHUNK):
                        off = g * GROUP + ci * CHUNK
                        nc.tensor.matmul(
                            sS[:, ci * CHUNK:(ci + 1) * CHUNK],
                            Wq[s][:, :],
                            X[:, off:off + CHUNK],
                            start=True, stop=True,
                        )
                    E = ep.tile([128, GROUP], fp16, tag="E")
                    nc.scalar.activation(E[:, :], sS[:, :], AF.Exp,
                                         scale=mc[:, 0:1])
                    for fr8 in range(F_PER_GROUP):
                        f = g * F_PER_GROUP + fr8              # global f
                        for vh in range(2):
                            nc.tensor.matmul(
                                sO[vh][:, 2 * f:2 * f + 2],
                                E[:, fr8 * V + vh * 128:fr8 * V + vh * 128 + 128],
                                Wamp[s][:, 2 * f:2 * f + 2],
                                start=True, stop=True,
                            )
                # drain psum -> sbuf -> DRAM out[v, f, 2s+b']
                for vh in range(2):
                    ob = cp.tile([128, 2 * F], fp32, tag=f"ob{vh}")
                    nc.vector.tensor_copy(ob[:, :], sO[vh][:, :])
                    nc.sync.dma_start(
                        bass.AP(tensor=d_out, offset=vh * 128 * F * B_LOC + 2 * s,
                                ap=[[F * B_LOC, 128], [B_LOC, F], [1, 2]]),
                        ob[:, :],
                    )

    nc.compile()
    return nc


def _get_nc():
    if "nc" not in _cache:
        _cache["nc"] = _build()
    return _cache["nc"]


def kernel(q2_obs_scaled, amplitude_logits, volumes, filters, sigma,
           _trace=False, _tmpdir=None):
    from concourse.bass_utils import run_bass_kernel_spmd

    nc = _get_nc()

    q = np.ascontiguousarray(np.asarray(q2_obs_scaled, dtype=np.float32))
    lg = np.asarray(amplitude_logits, dtype=np.float32).reshape(B, F, P)
    vol = np.ascontiguousarray(np.asarray(volumes, dtype=np.float32).reshape(V))
    fil = np.ascontiguousarray(np.asarray(filters, dtype=np.float32).reshape(F))
    sig = np.asarray(sigma, dtype=np.float32).reshape(1)

    in_maps = []
    for i in range(NCORES):
        bsl = slice(i * B_LOC, (i + 1) * B_LOC)
        lgc = lg[bsl]                                    # (B_LOC, F, P)
        in_maps.append({
            "q": np.ascontiguousarray(q[bsl].reshape(B_LOC * P)),
            "lf": np.ascontiguousarray(lgc.transpose(1, 0, 2)),   # (F,B_LOC,P)
            "lt": np.ascontiguousarray(
                lgc.transpose(0, 2, 1).reshape(B_LOC * P, F)),    # ((b,p),F)
            "vol": vol,
            "fil": fil,
            "sig": sig,
        })

    kw = {}
    if _trace:
        kw = {"trace": True, "tmpdir": _tmpdir}
    res = run_bass_kernel_spmd(nc, in_maps, core_ids=list(range(NCORES)), **kw)

    out = np.empty((B, V, F), dtype=np.float32)
    for i in range(NCORES):
        oc = res.results[i]["out"]                       # (V, F, B_LOC)
        out[i * B_LOC:(i + 1) * B_LOC] = oc.transpose(2, 0, 1)
    if _trace:
        return out, res
    return out


# revision 16
# speedup vs baseline: 2.2864x; 2.2864x over previous
"""Trainium2 Bass kernel for nn_ExtractionLayer.

metric[b,v,f] = sum_p amp[b,f,p] * exp(-c*(vol[v]*filt[f] - q[b,p])^2)
  amp = softmax_p(logits[b,f,p]),  c = 0.5/(sigma+0.001)^2

Sharding: data-parallel over batch B=32 -> 4 b's per core on 8 cores.

Per-core algorithm (2 "sets", each set = 2 b's = 128 (b,p) partition pairs):
  PE pass 1 : S[(b,p),(f,v)] = x^2 - 2qx + q^2 via K=3 matmul
              (lhsT rows {1,-2q,q^2} per (b,p) col, rhs rows {x^2,x,1})
  ACT pass  : E = exp(-c*S)  PSUM->SBUF (fp16), the dominant exp pass
  PE pass 2 : per (f, v-half): lhsT = E-slice (128,128) stationary,
              rhs = block-diag softmax weight pair (128,2) moving ->
              psum out (128 v, 2 b) -- partition-dense output
  drain     : DVE copy psum->SBUF, DMA -> out[v,f,b]; host -> [b,v,f]
"""

import sys

for _p in ("/opt/trn_rl_repo", "/root/.axon_site/_ro/trn_rl_repo"):
    if _p not in sys.path:
        sys.path.append(_p)

import numpy as np

B, V, F, P = 32, 256, 128, 64
NCORES = 8
B_LOC = B // NCORES          # 4 batches per core
NSETS = B_LOC // 2           # 2 sets of (2 b's x 64 p) = 128 partitions
NVF = V * F                  # 32768 (f-major: i = f*V + v)
CHUNK = 512                  # PE-1 matmul free dim (= 2 f's)
GROUP = 2048                 # ACT free dim (= 8 f's, 4 chunks, 4 PSUM banks)
NGROUPS = NVF // GROUP       # 16 per set
F_PER_GROUP = GROUP // V     # 8
F_PER_OTILE = 4              # psO tile covers 4 f's -> (2, 1024) = 2 banks

_cache: dict = {}


def _build():
    import concourse.tile as tile
    from concourse import bacc, mybir

    fp32 = mybir.dt.float32
    fp16 = mybir.dt.float16
    bf16 = mybir.dt.bfloat16
    AF = mybir.ActivationFunctionType
    OP = mybir.AluOpType
    import concourse.bass as bass

    nc = bacc.Bacc("TRN2", target_bir_lowering=False, debug=False,
                   num_devices=NCORES)

    d_q = nc.dram_tensor("q", [B_LOC * P], fp32, kind="ExternalInput")
    d_lf = nc.dram_tensor("lf", [F, B_LOC, P], fp32, kind="ExternalInput")
    d_lt = nc.dram_tensor("lt", [B_LOC * P, F], fp32, kind="ExternalInput")
    d_vol = nc.dram_tensor("vol", [V], fp32, kind="ExternalInput")
    d_fil = nc.dram_tensor("fil", [F], fp32, kind="ExternalInput")
    d_sig = nc.dram_tensor("sig", [1], fp32, kind="ExternalInput")
    # out[s, v, f, b'] -> contiguous DMA per (set, v-half); host interleaves
    d_out = nc.dram_tensor("out", [NSETS, V, F, 2], fp32, kind="ExternalOutput")
    d_zb = nc.dram_tensor("zb", [B_LOC * F], fp32)  # Zinv bounce, [b][f]

    with tile.TileContext(nc) as tc:
        with (
            tc.tile_pool(name="const", bufs=1) as cp,
            tc.tile_pool(name="ering", bufs=3) as ep,
            tc.tile_pool(name="psS", bufs=1, space=bass.MemorySpace.PSUM) as psS,
            tc.tile_pool(name="psO", bufs=2, space=bass.MemorySpace.PSUM) as psO,
        ):
            # ---- scalar constants from sigma, on all 128 partitions ----
            sigc = cp.tile([128, 1], fp32, tag="sigc")
            nc.sync.dma_start(
                sigc[:, :],
                bass.AP(tensor=d_sig, offset=0, ap=[[0, 128], [1, 1]]),
            )
            invc = cp.tile([128, 1], fp32, tag="invc")
            nc.vector.tensor_scalar_add(invc[:, :], sigc[:, :], 0.001)
            nc.vector.reciprocal(invc[:, :], invc[:, :])  # 1/(sig+.001)
            mc = cp.tile([128, 1], fp32, tag="mc")        # -c = -0.5*inv^2
            nc.vector.tensor_tensor(mc[:, :], invc[:, :], invc[:, :], OP.mult)
            nc.vector.tensor_scalar_mul(mc[:, :], mc[:, :], -0.5)

            # ---- X12 = bf16 split-precision rows for S = x^2 - 2qx + q^2 ----
            # 3-way hi/mid/lo bf16 splits keep ~fp32 accuracy at full bf16
            # matmul rate. Rows (K=12):
            #   0..2 : a_h a_m a_l  (a = x^2)        weights 1
            #   3..5 : b_h b_h b_h  (b = x)          weights w_h w_m w_l
            #   6..7 : b_m b_m                       weights w_h w_m
            #   8    : b_l                           weights w_h
            #   9..11: 1 1 1                         weights c_h c_m c_l
            # where w = -2q, c = q^2 per (b,p) column.
            filc = cp.tile([128, 1], fp32, tag="filc")
            nc.sync.dma_start(filc[:, :], d_fil.ap().rearrange("(f o) -> f o", o=1))
            volr = cp.tile([128, V], fp32, tag="volr")
            nc.sync.dma_start(
                volr[:, :], bass.AP(tensor=d_vol, offset=0, ap=[[0, 128], [1, V]])
            )
            x_ft = cp.tile([128, V], fp32, tag="x_ft")
            nc.vector.tensor_scalar(x_ft[:, :], volr[:, :], filc[:, 0:1], None,
                                    op0=OP.mult)
            xsq_ft = cp.tile([128, V], fp32, tag="xsq_ft")
            nc.vector.tensor_tensor(xsq_ft[:, :], x_ft[:, :], x_ft[:, :], OP.mult)
            ones_bf = cp.tile([128, V], bf16, tag="ones_bf")
            nc.vector.memset(ones_bf[:, :], 1.0)

            def split3(val32, pfx):
                """fp32 (128,V) -> three bf16 (128,V) tiles h+m+l ~= val."""
                h = cp.tile([128, V], bf16, tag=f"{pfx}h", name=f"{pfx}h")
                m = cp.tile([128, V], bf16, tag=f"{pfx}m", name=f"{pfx}m")
                l = cp.tile([128, V], bf16, tag=f"{pfx}l", name=f"{pfx}l")
                r1 = cp.tile([128, V], fp32, tag=f"{pfx}r1", name=f"{pfx}r1")
                r2 = cp.tile([128, V], fp32, tag=f"{pfx}r2", name=f"{pfx}r2")
                nc.vector.tensor_copy(h[:, :], val32[:, :])
                nc.vector.tensor_tensor(r1[:, :], val32[:, :], h[:, :], OP.subtract)
                nc.vector.tensor_copy(m[:, :], r1[:, :])
                nc.vector.tensor_tensor(r2[:, :], r1[:, :], m[:, :], OP.subtract)
                nc.vector.tensor_copy(l[:, :], r2[:, :])
                return h, m, l

            a_h, a_m, a_l = split3(xsq_ft, "a")
            b_h, b_m, b_l = split3(x_ft, "b")

            X = cp.tile([12, NVF], bf16, tag="X")
            for r, t in enumerate([a_h, a_m, a_l, b_h, b_h, b_h, b_m, b_m,
                                   b_l, ones_bf, ones_bf, ones_bf]):
                nc.sync.dma_start(X[r:r + 1, :], t[:, :])

            # ---- W_q per set: 12 bf16 rows (see X12 comment) ----
            q_row = cp.tile([1, B_LOC * P], fp32, tag="q_row")
            nc.sync.dma_start(q_row[:, :], d_q.ap())
            Wq = []
            # scratch rows at partition 0: fp32 w, c and residuals + bf16 outs
            wt32 = cp.tile([1, 128], fp32, tag="wt32")
            ct32 = cp.tile([1, 128], fp32, tag="ct32")
            res1 = cp.tile([1, 128], fp32, tag="res1")
            res2 = cp.tile([1, 128], fp32, tag="res2")
            hb = cp.tile([1, 128], bf16, tag="hb")

            def split3_row(val32, w, rows):
                """bf16-split val32 (1,128) into partitions `rows` of w."""
                src = val32
                for i, r in enumerate(rows):
                    nc.vector.tensor_copy(hb[:, :], src[:, :])
                    nc.sync.dma_start(w[r:r + 1, :], hb[:, :])
                    if i < len(rows) - 1:
                        dst = res1 if src is not res1 else res2
                        nc.vector.tensor_tensor(dst[:, :], src[:, :], hb[:, :],
                                                OP.subtract)
                        src = dst

            for s in range(NSETS):
                w = cp.tile([12, 128], bf16, tag=f"Wq{s}", name=f"Wq{s}")
                qs = q_row[0:1, s * 128:(s + 1) * 128]
                nc.vector.memset(w[0:3, :], 1.0)
                nc.vector.tensor_scalar_mul(wt32[:, :], qs, -2.0)
                nc.vector.tensor_tensor(ct32[:, :], qs, qs, OP.mult)
                split3_row(wt32, w, [3, 4, 5])    # w_h w_m w_l
                nc.sync.dma_start(w[6:7, :], w[3:4, :])   # w_h (vs b_m)
                nc.sync.dma_start(w[7:8, :], w[4:5, :])   # w_m (vs b_m)
                nc.sync.dma_start(w[8:9, :], w[3:4, :])   # w_h (vs b_l)
                split3_row(ct32, w, [9, 10, 11])  # c_h c_m c_l
                Wq.append(w)

            # ---- softmax denominators: Z[f, b] = sum_p exp(logits) ----
            lf_sb = cp.tile([128, B_LOC, P], fp32, tag="lf_sb")
            nc.sync.dma_start(lf_sb[:, :, :], d_lf.ap())
            el_f = cp.tile([128, B_LOC, P], fp32, tag="el_f")
            nc.scalar.activation(el_f[:, :, :], lf_sb[:, :, :], AF.Exp)
            Z = cp.tile([128, B_LOC], fp32, tag="Z")
            nc.vector.tensor_reduce(Z[:, :], el_f[:, :, :], mybir.AxisListType.X,
                                    OP.add)
            Zinv = cp.tile([128, B_LOC], fp32, tag="Zinv")
            nc.vector.reciprocal(Zinv[:, :], Z[:, :])
            # bounce transposed to [b][f] so reloads are contiguous rows
            nc.sync.dma_start(
                bass.AP(tensor=d_zb, offset=0, ap=[[1, 128], [128, B_LOC]]),
                Zinv[:, :],
            )

            # ---- W_amp per set: block-diag fp16 softmax weights ----
            # W_amp[k=(b,p), 2f+h] = amp[b,f,p] for k//64==h else 0
            Wamp = []
            for s in range(NSETS):
                lt_sb = cp.tile([128, F], fp32, tag=f"lt{s}")
                nc.sync.dma_start(lt_sb[:, :], d_lt.ap()[s * 128:(s + 1) * 128, :])
                elt = cp.tile([128, F], fp32, tag=f"elt{s}")
                nc.scalar.activation(elt[:, :], lt_sb[:, :], AF.Exp)
                zr = cp.tile([128, F], fp32, tag=f"zr{s}")
                for h in range(2):
                    nc.sync.dma_start(
                        zr[h * 64:(h + 1) * 64, :],
                        bass.AP(tensor=d_zb, offset=(2 * s + h) * F,
                                ap=[[0, 64], [1, F]]),
                    )
                w = cp.tile([128, 2 * F], fp16, tag=f"Wamp{s}")
                nc.vector.memset(w[:, :], 0.0)
                for h in range(2):
                    nc.vector.tensor_tensor(
                        w[h * 64:(h + 1) * 64, h:2 * F:2],
                        elt[h * 64:(h + 1) * 64, :],
                        zr[h * 64:(h + 1) * 64, :],
                        OP.mult,
                    )
                Wamp.append(w)

            # ---- main pipeline ----
            for s in range(NSETS):
                # psum out accumulators: [v(128), f(128)*b'(2)] per v-half
                sO = [psO.tile([128, 2 * F], fp32, tag=f"O{vh}", name=f"sO{vh}")
                      for vh in range(2)]
                for g in range(NGROUPS):
                    sS = psS.tile([128, GROUP], fp32, tag="S")
                    for ci in range(GROUP // CHUNK):
                        off = g * GROUP + ci * CHUNK
                        nc.tensor.matmul(
                            sS[:, ci * CHUNK:(ci + 1) * CHUNK],
                            Wq[s][:, :],
                            X[:, off:off + CHUNK],
                            start=True, stop=True,
                        )
                    E = ep.tile([128, GROUP], fp16, tag="E")
                    nc.scalar.activation(E[:, :], sS[:, :], AF.Exp,
                                         scale=mc[:, 0:1])
                    for fr8 in range(F_PER_GROUP):
                        f = g * F_PER_GROUP + fr8              # global f
                        for vh in range(2):
                            nc.tensor.matmul(
                                sO[vh][:, 2 * f:2 * f + 2],
                                E[:, fr8 * V + vh * 128:fr8 * V + vh * 128 + 128],
                                Wamp[s][:, 2 * f:2 * f + 2],
                                start=True, stop=True,
                            )
                # drain psum -> sbuf -> DRAM out[s, v, f, b'] (contiguous)
                for vh in range(2):
                    ob = cp.tile([128, 2 * F], fp32, tag=f"ob{vh}")
                    nc.vector.tensor_copy(ob[:, :], sO[vh][:, :])
                    nc.sync.dma_start(
                        d_out.ap()[s:s + 1, vh * 128:(vh + 1) * 128, :, :],
                        ob[:, :],
                    )

    nc.compile()
    return nc


def _get_nc():
    if "nc" not in _cache:
        _cache["nc"] = _build()
    return _cache["nc"]


def kernel(q2_obs_scaled, amplitude_logits, volumes, filters, sigma,
           _trace=False, _tmpdir=None):
    from concourse.bass_utils import run_bass_kernel_spmd

    nc = _get_nc()

    q = np.ascontiguousarray(np.asarray(q2_obs_scaled, dtype=np.float32))
    lg = np.asarray(amplitude_logits, dtype=np.float32).reshape(B, F, P)
    vol = np.ascontiguousarray(np.asarray(volumes, dtype=np.float32).reshape(V))
    fil = np.ascontiguousarray(np.asarray(filters, dtype=np.float32).reshape(F))
    sig = np.asarray(sigma, dtype=np.float32).reshape(1)

    in_maps = []
    for i in range(NCORES):
        bsl = slice(i * B_LOC, (i + 1) * B_LOC)
        lgc = lg[bsl]                                    # (B_LOC, F, P)
        in_maps.append({
            "q": np.ascontiguousarray(q[bsl].reshape(B_LOC * P)),
            "lf": np.ascontiguousarray(lgc.transpose(1, 0, 2)),   # (F,B_LOC,P)
            "lt": np.ascontiguousarray(
                lgc.transpose(0, 2, 1).reshape(B_LOC * P, F)),    # ((b,p),F)
            "vol": vol,
            "fil": fil,
            "sig": sig,
        })

    kw = {}
    if _trace:
        kw = {"trace": True, "tmpdir": _tmpdir}
    res = run_bass_kernel_spmd(nc, in_maps, core_ids=list(range(NCORES)), **kw)

    out = np.empty((B, V, F), dtype=np.float32)
    for i in range(NCORES):
        oc = res.results[i]["out"]                       # (NSETS, V, F, 2)
        for s in range(NSETS):
            for h in range(2):
                out[i * B_LOC + 2 * s + h] = oc[s, :, :, h]
    if _trace:
        return out, res
    return out


# revision 20
# speedup vs baseline: 3.3689x; 1.4735x over previous
"""Trainium2 Bass kernel for nn_ExtractionLayer.

metric[b,v,f] = sum_p amp[b,f,p] * exp(-c*(vol[v]*filt[f] - q[b,p])^2)
  amp = softmax_p(logits[b,f,p]),  c = 0.5/(sigma+0.001)^2

Sharding: data-parallel over batch B=32 -> 4 b's per core on 8 cores.

Per-core algorithm (2 "sets", each set = 2 b's = 128 (b,p) partition pairs):
  PE pass 1 : S[(b,p),(f,v)] = x^2 - 2qx + q^2 via K=3 matmul
              (lhsT rows {1,-2q,q^2} per (b,p) col, rhs rows {x^2,x,1})
  ACT pass  : E = exp(-c*S)  PSUM->SBUF (fp16), the dominant exp pass
  PE pass 2 : per (f, v-half): lhsT = E-slice (128,128) stationary,
              rhs = block-diag softmax weight pair (128,2) moving ->
              psum out (128 v, 2 b) -- partition-dense output
  drain     : DVE copy psum->SBUF, DMA -> out[v,f,b]; host -> [b,v,f]
"""

import sys

for _p in ("/opt/trn_rl_repo", "/root/.axon_site/_ro/trn_rl_repo"):
    if _p not in sys.path:
        sys.path.append(_p)

import numpy as np

B, V, F, P = 32, 256, 128, 64
NCORES = 8
B_LOC = B // NCORES          # 4 batches per core
NSETS = B_LOC // 2           # 2 sets of (2 b's x 64 p) = 128 partitions
NVF = V * F                  # 32768 (f-major: i = f*V + v)
GROUP = 1024                 # ACT free dim = PE-1 bf16 matmul N (2 PSUM banks)
NGROUPS = NVF // GROUP       # 32 per set
F_PER_GROUP = GROUP // V     # 4

_cache: dict = {}


def _build():
    import concourse.tile as tile
    from concourse import bacc, mybir

    fp32 = mybir.dt.float32
    fp16 = mybir.dt.float16
    bf16 = mybir.dt.bfloat16
    AF = mybir.ActivationFunctionType
    OP = mybir.AluOpType
    import concourse.bass as bass

    nc = bacc.Bacc("TRN2", target_bir_lowering=False, debug=False,
                   num_devices=NCORES)

    d_q = nc.dram_tensor("q", [B_LOC * P], fp32, kind="ExternalInput")
    d_lf = nc.dram_tensor("lf", [F, B_LOC, P], fp32, kind="ExternalInput")
    d_lt = nc.dram_tensor("lt", [B_LOC * P, F], fp32, kind="ExternalInput")
    d_vol = nc.dram_tensor("vol", [V], fp32, kind="ExternalInput")
    d_fil = nc.dram_tensor("fil", [F], fp32, kind="ExternalInput")
    d_sig = nc.dram_tensor("sig", [1], fp32, kind="ExternalInput")
    # out[s, v, f, b'] -> contiguous DMA per (set, v-half); host interleaves
    d_out = nc.dram_tensor("out", [NSETS, V, F, 2], fp32, kind="ExternalOutput")
    d_zb = nc.dram_tensor("zb", [B_LOC * F], fp32)  # Zinv bounce, [b][f]

    with tile.TileContext(nc) as tc:
        with (
            tc.tile_pool(name="const", bufs=1) as cp,
            tc.tile_pool(name="ering", bufs=3) as ep,
            tc.tile_pool(name="psS", bufs=2, space=bass.MemorySpace.PSUM) as psS,
            tc.tile_pool(name="psO", bufs=1, space=bass.MemorySpace.PSUM) as psO,
        ):
            # round-robin DMA issue over engines that sit mostly idle
            dmaeng = [nc.sync, nc.gpsimd]
            dmactr = [0]

            def dma(dst, src):
                e = dmaeng[dmactr[0] % len(dmaeng)]
                dmactr[0] += 1
                e.dma_start(dst, src)
            # ---- scalar constants from sigma, on all 128 partitions ----
            sigc = cp.tile([128, 1], fp32, tag="sigc")
            dma(sigc[:, :], bass.AP(tensor=d_sig, offset=0, ap=[[0, 128], [1, 1]]))
            invc = cp.tile([128, 1], fp32, tag="invc")
            nc.vector.tensor_scalar_add(invc[:, :], sigc[:, :], 0.001)
            nc.vector.reciprocal(invc[:, :], invc[:, :])  # 1/(sig+.001)
            mc = cp.tile([128, 1], fp32, tag="mc")        # -c = -0.5*inv^2
            nc.vector.tensor_tensor(mc[:, :], invc[:, :], invc[:, :], OP.mult)
            nc.vector.tensor_scalar_mul(mc[:, :], mc[:, :], -0.5)

            # ---- X12 = bf16 split-precision rows for S = x^2 - 2qx + q^2 ----
            # 3-way hi/mid/lo bf16 splits keep ~fp32 accuracy at full bf16
            # matmul rate. Rows (K=12):
            #   0..2 : a_h a_m a_l  (a = x^2)        weights 1
            #   3..5 : b_h b_h b_h  (b = x)          weights w_h w_m w_l
            #   6..7 : b_m b_m                       weights w_h w_m
            #   8    : b_l                           weights w_h
            #   9..11: 1 1 1                         weights c_h c_m c_l
            # where w = -2q, c = q^2 per (b,p) column.
            filc = cp.tile([128, 1], fp32, tag="filc")
            dma(filc[:, :], d_fil.ap().rearrange("(f o) -> f o", o=1))
            volr = cp.tile([128, V], fp32, tag="volr")
            dma(volr[:, :], bass.AP(tensor=d_vol, offset=0, ap=[[0, 128], [1, V]]))
            x_ft = cp.tile([128, V], fp32, tag="x_ft")
            nc.vector.tensor_scalar(x_ft[:, :], volr[:, :], filc[:, 0:1], None,
                                    op0=OP.mult)
            xsq_ft = cp.tile([128, V], fp32, tag="xsq_ft")
            nc.vector.tensor_tensor(xsq_ft[:, :], x_ft[:, :], x_ft[:, :], OP.mult)
            ones_bf = cp.tile([128, V], bf16, tag="ones_bf")
            nc.vector.memset(ones_bf[:, :], 1.0)

            def split3(val32, pfx):
                """fp32 (128,V) -> three bf16 (128,V) tiles h+m+l ~= val."""
                h = cp.tile([128, V], bf16, tag=f"{pfx}h", name=f"{pfx}h")
                m = cp.tile([128, V], bf16, tag=f"{pfx}m", name=f"{pfx}m")
                l = cp.tile([128, V], bf16, tag=f"{pfx}l", name=f"{pfx}l")
                r1 = cp.tile([128, V], fp32, tag=f"{pfx}r1", name=f"{pfx}r1")
                r2 = cp.tile([128, V], fp32, tag=f"{pfx}r2", name=f"{pfx}r2")
                nc.vector.tensor_copy(h[:, :], val32[:, :])
                nc.vector.tensor_tensor(r1[:, :], val32[:, :], h[:, :], OP.subtract)
                nc.vector.tensor_copy(m[:, :], r1[:, :])
                nc.vector.tensor_tensor(r2[:, :], r1[:, :], m[:, :], OP.subtract)
                nc.vector.tensor_copy(l[:, :], r2[:, :])
                return h, m, l

            a_h, a_m, a_l = split3(xsq_ft, "a")
            b_h, b_m, b_l = split3(x_ft, "b")

            X = cp.tile([12, NVF], bf16, tag="X")
            for r, t in enumerate([a_h, a_m, a_l, b_h, b_h, b_h, b_m, b_m,
                                   b_l, ones_bf, ones_bf, ones_bf]):
                dma(X[r:r + 1, :], t[:, :])

            # ---- W_q per set: 12 bf16 rows (see X12 comment) ----
            q_row = cp.tile([1, B_LOC * P], fp32, tag="q_row")
            dma(q_row[:, :], d_q.ap())
            Wq = []
            # scratch rows at partition 0: fp32 w, c and residuals + bf16 outs
            wt32 = cp.tile([1, 128], fp32, tag="wt32")
            ct32 = cp.tile([1, 128], fp32, tag="ct32")
            res1 = cp.tile([1, 128], fp32, tag="res1")
            res2 = cp.tile([1, 128], fp32, tag="res2")

            def split3_row(val32, srow, cols):
                """bf16-split val32 (1,128) into 128-col slices of srow."""
                cur = val32
                for i, cidx in enumerate(cols):
                    hb = srow[0:1, cidx * 128:(cidx + 1) * 128]
                    nc.vector.tensor_copy(hb, cur[:, :])
                    if i < len(cols) - 1:
                        dst = res1 if cur is not res1 else res2
                        nc.vector.tensor_tensor(dst[:, :], cur[:, :], hb,
                                                OP.subtract)
                        cur = dst

            for s in range(NSETS):
                # all 12 rows packed in one (1, 12*128) bf16 scratch, one DMA
                srow = cp.tile([1, 12 * 128], bf16, tag=f"srow{s}",
                               name=f"srow{s}")
                w = cp.tile([12, 128], bf16, tag=f"Wq{s}", name=f"Wq{s}")
                qs = q_row[0:1, s * 128:(s + 1) * 128]
                nc.vector.memset(srow[0:1, 0:3 * 128], 1.0)
                nc.vector.tensor_scalar_mul(wt32[:, :], qs, -2.0)
                nc.vector.tensor_tensor(ct32[:, :], qs, qs, OP.mult)
                split3_row(wt32, srow, [3, 4, 5])   # w_h w_m w_l
                nc.vector.tensor_copy(srow[0:1, 6 * 128:7 * 128],
                                      srow[0:1, 3 * 128:4 * 128])  # w_h
                nc.vector.tensor_copy(srow[0:1, 7 * 128:8 * 128],
                                      srow[0:1, 4 * 128:5 * 128])  # w_m
                nc.vector.tensor_copy(srow[0:1, 8 * 128:9 * 128],
                                      srow[0:1, 3 * 128:4 * 128])  # w_h
                split3_row(ct32, srow, [9, 10, 11])  # c_h c_m c_l
                dma(w[:, :], srow[0:1, :])
                Wq.append(w)

            # ---- softmax denominators: Z[f, b] = sum_p exp(logits) ----
            lf_sb = cp.tile([128, B_LOC, P], fp32, tag="lf_sb")
            dma(lf_sb[:, :, :], d_lf.ap())
            el_f = cp.tile([128, B_LOC, P], fp32, tag="el_f")
            nc.scalar.activation(el_f[:, :, :], lf_sb[:, :, :], AF.Exp)
            Z = cp.tile([128, B_LOC], fp32, tag="Z")
            nc.vector.tensor_reduce(Z[:, :], el_f[:, :, :], mybir.AxisListType.X,
                                    OP.add)
            Zinv = cp.tile([128, B_LOC], fp32, tag="Zinv")
            nc.vector.reciprocal(Zinv[:, :], Z[:, :])
            # bounce transposed to [b][f] so reloads are contiguous rows
            dma(bass.AP(tensor=d_zb, offset=0, ap=[[1, 128], [128, B_LOC]]),
                Zinv[:, :])

            # ---- W_amp per set: block-diag fp16 softmax weights ----
            # W_amp[k=(b,p), 2f+h] = amp[b,f,p] for k//64==h else 0
            Wamp = []
            for s in range(NSETS):
                lt_sb = cp.tile([128, F], fp32, tag=f"lt{s}", name=f"lt{s}")
                dma(lt_sb[:, :], d_lt.ap()[s * 128:(s + 1) * 128, :])
                elt = cp.tile([128, F], fp32, tag=f"elt{s}")
                nc.scalar.activation(elt[:, :], lt_sb[:, :], AF.Exp)
                zr = cp.tile([128, F], fp32, tag=f"zr{s}", name=f"zr{s}")
                for h in range(2):
                    dma(zr[h * 64:(h + 1) * 64, :],
                        bass.AP(tensor=d_zb, offset=(2 * s + h) * F,
                                ap=[[0, 64], [1, F]]))
                w = cp.tile([128, 2 * F], fp16, tag=f"Wamp{s}", name=f"Wamp{s}")
                nc.vector.memset(w[:, :], 0.0)
                for h in range(2):
                    nc.vector.tensor_tensor(
                        w[h * 64:(h + 1) * 64, h:2 * F:2],
                        elt[h * 64:(h + 1) * 64, :],
                        zr[h * 64:(h + 1) * 64, :],
                        OP.mult,
                    )
                Wamp.append(w)

            # ---- main pipeline ----
            # single psum out accumulator, cols [(s*2+vh)*256 + 2*f + b']
            sO = psO.tile([128, 4 * 2 * F], fp32, tag="O", name="sO")
            for s in range(NSETS):
                for g in range(NGROUPS):
                    sS = psS.tile([128, GROUP], fp32, tag="S", name="sS")
                    for ci in range(2):
                        off = g * GROUP + ci * 512
                        nc.tensor.matmul(
                            sS[:, ci * 512:(ci + 1) * 512],
                            Wq[s][:, :],
                            X[:, off:off + 512],
                            start=True, stop=True,
                        )
                    E = ep.tile([128, GROUP], fp16, tag="E", name="E")
                    nc.scalar.activation(E[:, :], sS[:, :], AF.Exp,
                                         scale=mc[:, 0:1])
                    for fr in range(F_PER_GROUP):
                        f = g * F_PER_GROUP + fr               # global f
                        for vh in range(2):
                            base = (s * 2 + vh) * 2 * F
                            nc.tensor.matmul(
                                sO[:, base + 2 * f:base + 2 * f + 2],
                                E[:, fr * V + vh * 128:fr * V + vh * 128 + 128],
                                Wamp[s][:, 2 * f:2 * f + 2],
                                start=True, stop=True,
                            )
                # drain psum -> sbuf -> DRAM out[s, v, f, b'] (contiguous)
                for vh in range(2):
                    ob = cp.tile([128, 2 * F], fp32, tag=f"ob{vh}",
                                 name=f"ob{vh}")
                    base = (s * 2 + vh) * 2 * F
                    nc.vector.tensor_copy(ob[:, :], sO[:, base:base + 2 * F])
                    nc.sync.dma_start(
                        d_out.ap()[s:s + 1, vh * 128:(vh + 1) * 128, :, :],
                        ob[:, :],
                    )

    nc.compile()
    return nc


def _get_nc():
    if "nc" not in _cache:
        _cache["nc"] = _build()
    return _cache["nc"]


def kernel(q2_obs_scaled, amplitude_logits, volumes, filters, sigma,
           _trace=False, _tmpdir=None):
    from concourse.bass_utils import run_bass_kernel_spmd

    nc = _get_nc()

    q = np.ascontiguousarray(np.asarray(q2_obs_scaled, dtype=np.float32))
    lg = np.asarray(amplitude_logits, dtype=np.float32).reshape(B, F, P)
    vol = np.ascontiguousarray(np.asarray(volumes, dtype=np.float32).reshape(V))
    fil = np.ascontiguousarray(np.asarray(filters, dtype=np.float32).reshape(F))
    sig = np.asarray(sigma, dtype=np.float32).reshape(1)

    in_maps = []
    for i in range(NCORES):
        bsl = slice(i * B_LOC, (i + 1) * B_LOC)
        lgc = lg[bsl]                                    # (B_LOC, F, P)
        in_maps.append({
            "q": np.ascontiguousarray(q[bsl].reshape(B_LOC * P)),
            "lf": np.ascontiguousarray(lgc.transpose(1, 0, 2)),   # (F,B_LOC,P)
            "lt": np.ascontiguousarray(
                lgc.transpose(0, 2, 1).reshape(B_LOC * P, F)),    # ((b,p),F)
            "vol": vol,
            "fil": fil,
            "sig": sig,
        })

    kw = {}
    if _trace:
        kw = {"trace": True, "tmpdir": _tmpdir}
    res = run_bass_kernel_spmd(nc, in_maps, core_ids=list(range(NCORES)), **kw)

    out = np.empty((B, V, F), dtype=np.float32)
    for i in range(NCORES):
        oc = res.results[i]["out"]                       # (NSETS, V, F, 2)
        for s in range(NSETS):
            for h in range(2):
                out[i * B_LOC + 2 * s + h] = oc[s, :, :, h]
    if _trace:
        return out, res
    return out


# revision 22
# speedup vs baseline: 3.6788x; 1.0920x over previous
"""Trainium2 Bass kernel for nn_ExtractionLayer.

metric[b,v,f] = sum_p amp[b,f,p] * exp(-c*(vol[v]*filt[f] - q[b,p])^2)
  amp = softmax_p(logits[b,f,p]),  c = 0.5/(sigma+0.001)^2

Sharding: data-parallel over batch B=32 -> 4 b's per core on 8 cores.

Per-core algorithm (2 "sets", each set = 2 b's = 128 (b,p) partition pairs):
  PE pass 1 : S[(b,p),(f,v)] = x^2 - 2qx + q^2 via K=3 matmul
              (lhsT rows {1,-2q,q^2} per (b,p) col, rhs rows {x^2,x,1})
  ACT pass  : E = exp(-c*S)  PSUM->SBUF (fp16), the dominant exp pass
  PE pass 2 : per (f, v-half): lhsT = E-slice (128,128) stationary,
              rhs = block-diag softmax weight pair (128,2) moving ->
              psum out (128 v, 2 b) -- partition-dense output
  drain     : DVE copy psum->SBUF, DMA -> out[v,f,b]; host -> [b,v,f]
"""

import sys

for _p in ("/opt/trn_rl_repo", "/root/.axon_site/_ro/trn_rl_repo"):
    if _p not in sys.path:
        sys.path.append(_p)

import numpy as np

B, V, F, P = 32, 256, 128, 64
NCORES = 8
B_LOC = B // NCORES          # 4 batches per core
NSETS = B_LOC // 2           # 2 sets of (2 b's x 64 p) = 128 partitions
NVF = V * F                  # 32768 (f-major: i = f*V + v)
GROUP = 1024                 # ACT free dim = PE-1 bf16 matmul N (2 PSUM banks)
NGROUPS = NVF // GROUP       # 32 per set
F_PER_GROUP = GROUP // V     # 4

_cache: dict = {}


def _build():
    import concourse.tile as tile
    from concourse import bacc, mybir

    fp32 = mybir.dt.float32
    fp16 = mybir.dt.float16
    bf16 = mybir.dt.bfloat16
    AF = mybir.ActivationFunctionType
    OP = mybir.AluOpType
    import concourse.bass as bass

    nc = bacc.Bacc("TRN2", target_bir_lowering=False, debug=False,
                   num_devices=NCORES)

    d_q = nc.dram_tensor("q", [B_LOC * P], fp32, kind="ExternalInput")
    d_lf = nc.dram_tensor("lf", [F, B_LOC, P], fp32, kind="ExternalInput")
    d_lt = nc.dram_tensor("lt", [B_LOC * P, F], fp32, kind="ExternalInput")
    d_vol = nc.dram_tensor("vol", [V], fp32, kind="ExternalInput")
    d_fil = nc.dram_tensor("fil", [F], fp32, kind="ExternalInput")
    d_sig = nc.dram_tensor("sig", [1], fp32, kind="ExternalInput")
    # out[s, v, f, b'] -> contiguous DMA per (set, v-half); host interleaves
    d_out = nc.dram_tensor("out", [NSETS, V, F, 2], fp32, kind="ExternalOutput")
    d_zb = nc.dram_tensor("zb", [B_LOC * F], fp32)  # Zinv bounce, [b][f]

    with tile.TileContext(nc) as tc:
        with (
            tc.tile_pool(name="const", bufs=1) as cp,
            tc.tile_pool(name="ering", bufs=3) as ep,
            tc.tile_pool(name="psS", bufs=2, space=bass.MemorySpace.PSUM) as psS,
            tc.tile_pool(name="psO", bufs=1, space=bass.MemorySpace.PSUM) as psO,
        ):
            # round-robin DMA issue over engines that sit mostly idle
            dmaeng = [nc.sync, nc.gpsimd]
            dmactr = [0]

            def dma(dst, src):
                e = dmaeng[dmactr[0] % len(dmaeng)]
                dmactr[0] += 1
                e.dma_start(dst, src)

            # setup-only rotation may also use the ACT engine's DMA port
            dmaeng3 = [nc.sync, nc.gpsimd, nc.scalar]

            def dma3(dst, src):
                e = dmaeng3[dmactr[0] % len(dmaeng3)]
                dmactr[0] += 1
                e.dma_start(dst, src)

            # ---- scalar constants from sigma, on all 128 partitions ----
            sigc = cp.tile([128, 1], fp32, tag="sigc")
            dma(sigc[:, :], bass.AP(tensor=d_sig, offset=0, ap=[[0, 128], [1, 1]]))
            invc = cp.tile([128, 1], fp32, tag="invc")
            nc.vector.tensor_scalar_add(invc[:, :], sigc[:, :], 0.001)
            nc.vector.reciprocal(invc[:, :], invc[:, :])  # 1/(sig+.001)
            mc = cp.tile([128, 1], fp32, tag="mc")        # -c = -0.5*inv^2
            nc.vector.tensor_tensor(mc[:, :], invc[:, :], invc[:, :], OP.mult)
            nc.vector.tensor_scalar_mul(mc[:, :], mc[:, :], -0.5)

            # ---- X12 = bf16 split-precision rows for S = x^2 - 2qx + q^2 ----
            # 3-way hi/mid/lo bf16 splits keep ~fp32 accuracy at full bf16
            # matmul rate. Rows (K=12):
            #   0..2 : a_h a_m a_l  (a = x^2)        weights 1
            #   3..5 : b_h b_h b_h  (b = x)          weights w_h w_m w_l
            #   6..7 : b_m b_m                       weights w_h w_m
            #   8    : b_l                           weights w_h
            #   9..11: 1 1 1                         weights c_h c_m c_l
            # where w = -2q, c = q^2 per (b,p) column.
            filc = cp.tile([128, 1], fp32, tag="filc")
            dma(filc[:, :], d_fil.ap().rearrange("(f o) -> f o", o=1))
            volr = cp.tile([128, V], fp32, tag="volr")
            dma(volr[:, :], bass.AP(tensor=d_vol, offset=0, ap=[[0, 128], [1, V]]))
            x_ft = cp.tile([128, V], fp32, tag="x_ft")
            nc.vector.tensor_scalar(x_ft[:, :], volr[:, :], filc[:, 0:1], None,
                                    op0=OP.mult)
            xsq_ft = cp.tile([128, V], fp32, tag="xsq_ft")
            nc.vector.tensor_tensor(xsq_ft[:, :], x_ft[:, :], x_ft[:, :], OP.mult)
            ones_bf = cp.tile([128, V], bf16, tag="ones_bf")
            nc.vector.memset(ones_bf[:, :], 1.0)

            def split3(val32, pfx):
                """fp32 (128,V) -> three bf16 (128,V) tiles h+m+l ~= val."""
                h = cp.tile([128, V], bf16, tag=f"{pfx}h", name=f"{pfx}h")
                m = cp.tile([128, V], bf16, tag=f"{pfx}m", name=f"{pfx}m")
                l = cp.tile([128, V], bf16, tag=f"{pfx}l", name=f"{pfx}l")
                r1 = cp.tile([128, V], fp32, tag=f"{pfx}r1", name=f"{pfx}r1")
                r2 = cp.tile([128, V], fp32, tag=f"{pfx}r2", name=f"{pfx}r2")
                nc.vector.tensor_copy(h[:, :], val32[:, :])
                nc.vector.tensor_tensor(r1[:, :], val32[:, :], h[:, :], OP.subtract)
                nc.vector.tensor_copy(m[:, :], r1[:, :])
                nc.vector.tensor_tensor(r2[:, :], r1[:, :], m[:, :], OP.subtract)
                nc.vector.tensor_copy(l[:, :], r2[:, :])
                return h, m, l

            a_h, a_m, a_l = split3(xsq_ft, "a")
            b_h, b_m, b_l = split3(x_ft, "b")

            # X row-writes hit a single SBUF partition each (~2.8 GB/s), so
            # split into column pieces: PE-1 starts after piece 0 while the
            # rest streams in (range-granular Tile deps).
            X = cp.tile([12, NVF], bf16, tag="X")
            NPIECE = 4
            FPP = 128 // NPIECE                      # f's per piece
            srcs = [a_h, a_m, a_l, b_h, b_h, b_h, b_m, b_m,
                    b_l, ones_bf, ones_bf, ones_bf]
            for p in range(NPIECE):
                for r, t in enumerate(srcs):
                    dma3(X[r:r + 1, p * FPP * V:(p + 1) * FPP * V],
                         t[p * FPP:(p + 1) * FPP, :])

            # ---- W_q per set: 12 bf16 rows (see X12 comment) ----
            q_row = cp.tile([1, B_LOC * P], fp32, tag="q_row")
            dma(q_row[:, :], d_q.ap())
            Wq = []
            # scratch rows at partition 0: fp32 w, c and residuals + bf16 outs
            wt32 = cp.tile([1, 128], fp32, tag="wt32")
            ct32 = cp.tile([1, 128], fp32, tag="ct32")
            res1 = cp.tile([1, 128], fp32, tag="res1")
            res2 = cp.tile([1, 128], fp32, tag="res2")

            def split3_row(val32, srow, cols):
                """bf16-split val32 (1,128) into 128-col slices of srow."""
                cur = val32
                for i, cidx in enumerate(cols):
                    hb = srow[0:1, cidx * 128:(cidx + 1) * 128]
                    nc.vector.tensor_copy(hb, cur[:, :])
                    if i < len(cols) - 1:
                        dst = res1 if cur is not res1 else res2
                        nc.vector.tensor_tensor(dst[:, :], cur[:, :], hb,
                                                OP.subtract)
                        cur = dst

            for s in range(NSETS):
                # all 12 rows packed in one (1, 12*128) bf16 scratch, one DMA
                srow = cp.tile([1, 12 * 128], bf16, tag=f"srow{s}",
                               name=f"srow{s}")
                w = cp.tile([12, 128], bf16, tag=f"Wq{s}", name=f"Wq{s}")
                qs = q_row[0:1, s * 128:(s + 1) * 128]
                nc.vector.memset(srow[0:1, 0:3 * 128], 1.0)
                nc.vector.tensor_scalar_mul(wt32[:, :], qs, -2.0)
                nc.vector.tensor_tensor(ct32[:, :], qs, qs, OP.mult)
                split3_row(wt32, srow, [3, 4, 5])   # w_h w_m w_l
                nc.vector.tensor_copy(srow[0:1, 6 * 128:7 * 128],
                                      srow[0:1, 3 * 128:4 * 128])  # w_h
                nc.vector.tensor_copy(srow[0:1, 7 * 128:8 * 128],
                                      srow[0:1, 4 * 128:5 * 128])  # w_m
                nc.vector.tensor_copy(srow[0:1, 8 * 128:9 * 128],
                                      srow[0:1, 3 * 128:4 * 128])  # w_h
                split3_row(ct32, srow, [9, 10, 11])  # c_h c_m c_l
                dma(w[:, :], srow[0:1, :])
                Wq.append(w)

            # ---- softmax denominators: Z[f, b] = sum_p exp(logits) ----
            lf_sb = cp.tile([128, B_LOC, P], fp32, tag="lf_sb")
            dma(lf_sb[:, :, :], d_lf.ap())
            el_f = cp.tile([128, B_LOC, P], fp32, tag="el_f")
            nc.scalar.activation(el_f[:, :, :], lf_sb[:, :, :], AF.Exp)
            Z = cp.tile([128, B_LOC], fp32, tag="Z")
            nc.vector.tensor_reduce(Z[:, :], el_f[:, :, :], mybir.AxisListType.X,
                                    OP.add)
            Zinv = cp.tile([128, B_LOC], fp32, tag="Zinv")
            nc.vector.reciprocal(Zinv[:, :], Z[:, :])
            # bounce transposed to [b][f] so reloads are contiguous rows
            dma(bass.AP(tensor=d_zb, offset=0, ap=[[1, 128], [128, B_LOC]]),
                Zinv[:, :])

            # ---- W_amp per set: block-diag fp16 softmax weights ----
            # W_amp[k=(b,p), 2f+h] = amp[b,f,p] for k//64==h else 0
            Wamp = []
            for s in range(NSETS):
                lt_sb = cp.tile([128, F], fp32, tag=f"lt{s}", name=f"lt{s}")
                dma(lt_sb[:, :], d_lt.ap()[s * 128:(s + 1) * 128, :])
                elt = cp.tile([128, F], fp32, tag=f"elt{s}")
                nc.scalar.activation(elt[:, :], lt_sb[:, :], AF.Exp)
                zr = cp.tile([128, F], fp32, tag=f"zr{s}", name=f"zr{s}")
                for h in range(2):
                    dma(zr[h * 64:(h + 1) * 64, :],
                        bass.AP(tensor=d_zb, offset=(2 * s + h) * F,
                                ap=[[0, 64], [1, F]]))
                w = cp.tile([128, 2 * F], fp16, tag=f"Wamp{s}", name=f"Wamp{s}")
                nc.vector.memset(w[:, :], 0.0)
                for h in range(2):
                    nc.vector.tensor_tensor(
                        w[h * 64:(h + 1) * 64, h:2 * F:2],
                        elt[h * 64:(h + 1) * 64, :],
                        zr[h * 64:(h + 1) * 64, :],
                        OP.mult,
                    )
                Wamp.append(w)

            # ---- main pipeline ----
            # single psum out accumulator, cols [(s*2+vh)*256 + 2*f + b']
            sO = psO.tile([128, 4 * 2 * F], fp32, tag="O", name="sO")
            for s in range(NSETS):
                for g in range(NGROUPS):
                    sS = psS.tile([128, GROUP], fp32, tag="S", name="sS")
                    for ci in range(2):
                        off = g * GROUP + ci * 512
                        nc.tensor.matmul(
                            sS[:, ci * 512:(ci + 1) * 512],
                            Wq[s][:, :],
                            X[:, off:off + 512],
                            start=True, stop=True,
                        )
                    E = ep.tile([128, GROUP], fp16, tag="E", name="E")
                    nc.scalar.activation(E[:, :], sS[:, :], AF.Exp,
                                         scale=mc[:, 0:1])
                    for fr in range(F_PER_GROUP):
                        f = g * F_PER_GROUP + fr               # global f
                        for vh in range(2):
                            base = (s * 2 + vh) * 2 * F
                            nc.tensor.matmul(
                                sO[:, base + 2 * f:base + 2 * f + 2],
                                E[:, fr * V + vh * 128:fr * V + vh * 128 + 128],
                                Wamp[s][:, 2 * f:2 * f + 2],
                                start=True, stop=True,
                            )
                # drain psum -> sbuf -> DRAM out[s, v, f, b'] (contiguous)
                for vh in range(2):
                    ob = cp.tile([128, 2 * F], fp32, tag=f"ob{vh}",
                                 name=f"ob{vh}")
                    base = (s * 2 + vh) * 2 * F
                    nc.vector.tensor_copy(ob[:, :], sO[:, base:base + 2 * F])
                    nc.sync.dma_start(
                        d_out.ap()[s:s + 1, vh * 128:(vh + 1) * 128, :, :],
                        ob[:, :],
                    )

    nc.compile()
    return nc


def _get_nc():
    if "nc" not in _cache:
        _cache["nc"] = _build()
    return _cache["nc"]


def kernel(q2_obs_scaled, amplitude_logits, volumes, filters, sigma,
           _trace=False, _tmpdir=None):
    from concourse.bass_utils import run_bass_kernel_spmd

    nc = _get_nc()

    q = np.ascontiguousarray(np.asarray(q2_obs_scaled, dtype=np.float32))
    lg = np.asarray(amplitude_logits, dtype=np.float32).reshape(B, F, P)
    vol = np.ascontiguousarray(np.asarray(volumes, dtype=np.float32).reshape(V))
    fil = np.ascontiguousarray(np.asarray(filters, dtype=np.float32).reshape(F))
    sig = np.asarray(sigma, dtype=np.float32).reshape(1)

    in_maps = []
    for i in range(NCORES):
        bsl = slice(i * B_LOC, (i + 1) * B_LOC)
        lgc = lg[bsl]                                    # (B_LOC, F, P)
        in_maps.append({
            "q": np.ascontiguousarray(q[bsl].reshape(B_LOC * P)),
            "lf": np.ascontiguousarray(lgc.transpose(1, 0, 2)),   # (F,B_LOC,P)
            "lt": np.ascontiguousarray(
                lgc.transpose(0, 2, 1).reshape(B_LOC * P, F)),    # ((b,p),F)
            "vol": vol,
            "fil": fil,
            "sig": sig,
        })

    kw = {}
    if _trace:
        kw = {"trace": True, "tmpdir": _tmpdir}
    res = run_bass_kernel_spmd(nc, in_maps, core_ids=list(range(NCORES)), **kw)

    out = np.empty((B, V, F), dtype=np.float32)
    for i in range(NCORES):
        oc = res.results[i]["out"]                       # (NSETS, V, F, 2)
        for s in range(NSETS):
            for h in range(2):
                out[i * B_LOC + 2 * s + h] = oc[s, :, :, h]
    if _trace:
        return out, res
    return out


# revision 23
# speedup vs baseline: 3.8409x; 1.0441x over previous
"""Trainium2 Bass kernel for nn_ExtractionLayer.

metric[b,v,f] = sum_p amp[b,f,p] * exp(-c*(vol[v]*filt[f] - q[b,p])^2)
  amp = softmax_p(logits[b,f,p]),  c = 0.5/(sigma+0.001)^2

Sharding: data-parallel over batch B=32 -> 4 b's per core on 8 cores.

Per-core algorithm (2 "sets", each set = 2 b's = 128 (b,p) partition pairs):
  PE pass 1 : S[(b,p),(f,v)] = x^2 - 2qx + q^2 via K=3 matmul
              (lhsT rows {1,-2q,q^2} per (b,p) col, rhs rows {x^2,x,1})
  ACT pass  : E = exp(-c*S)  PSUM->SBUF (fp16), the dominant exp pass
  PE pass 2 : per (f, v-half): lhsT = E-slice (128,128) stationary,
              rhs = block-diag softmax weight pair (128,2) moving ->
              psum out (128 v, 2 b) -- partition-dense output
  drain     : DVE copy psum->SBUF, DMA -> out[v,f,b]; host -> [b,v,f]
"""

import sys

for _p in ("/opt/trn_rl_repo", "/root/.axon_site/_ro/trn_rl_repo"):
    if _p not in sys.path:
        sys.path.append(_p)

import numpy as np

B, V, F, P = 32, 256, 128, 64
NCORES = 8
B_LOC = B // NCORES          # 4 batches per core
NSETS = B_LOC // 2           # 2 sets of (2 b's x 64 p) = 128 partitions
NVF = V * F                  # 32768 (f-major: i = f*V + v)
GROUP = 1536                 # ACT free dim (3 PSUM banks); last group ragged
# per-set group list: (start_col, n_cols), n_cols multiple of V
GROUPS = []
_c0 = 0
while _c0 < NVF:
    GROUPS.append((_c0, min(GROUP, NVF - _c0)))
    _c0 += GROUP

_cache: dict = {}


def _build(minus_c):
    import concourse.tile as tile
    from concourse import bacc, mybir

    fp32 = mybir.dt.float32
    fp16 = mybir.dt.float16
    bf16 = mybir.dt.bfloat16
    AF = mybir.ActivationFunctionType
    OP = mybir.AluOpType
    import concourse.bass as bass

    nc = bacc.Bacc("TRN2", target_bir_lowering=False, debug=False,
                   num_devices=NCORES)

    d_q = nc.dram_tensor("q", [B_LOC * P], fp32, kind="ExternalInput")
    d_lf = nc.dram_tensor("lf", [F, B_LOC, P], fp32, kind="ExternalInput")
    d_lt = nc.dram_tensor("lt", [B_LOC * P, F], fp32, kind="ExternalInput")
    d_vol = nc.dram_tensor("vol", [V], fp32, kind="ExternalInput")
    d_fil = nc.dram_tensor("fil", [F], fp32, kind="ExternalInput")
    d_sig = nc.dram_tensor("sig", [1], fp32, kind="ExternalInput")
    # out[s, v, f, b'] -> contiguous DMA per (set, v-half); host interleaves
    d_out = nc.dram_tensor("out", [NSETS, V, F, 2], fp32, kind="ExternalOutput")
    d_zb = nc.dram_tensor("zb", [B_LOC * F], fp32)  # Zinv bounce, [b][f]

    with tile.TileContext(nc) as tc:
        with (
            tc.tile_pool(name="const", bufs=1) as cp,
            tc.tile_pool(name="ering", bufs=3) as ep,
            tc.tile_pool(name="psS", bufs=2, space=bass.MemorySpace.PSUM) as psS,
            tc.tile_pool(name="psO", bufs=1, space=bass.MemorySpace.PSUM) as psO,
        ):
            # round-robin DMA issue over engines that sit mostly idle
            dmaeng = [nc.sync, nc.gpsimd]
            dmactr = [0]

            def dma(dst, src):
                e = dmaeng[dmactr[0] % len(dmaeng)]
                dmactr[0] += 1
                e.dma_start(dst, src)

            # setup-only rotation may also use the ACT engine's DMA port
            dmaeng3 = [nc.sync, nc.gpsimd, nc.scalar]

            def dma3(dst, src):
                e = dmaeng3[dmactr[0] % len(dmaeng3)]
                dmactr[0] += 1
                e.dma_start(dst, src)

            # ---- all input loads first: no deps, fire immediately ----
            q_row = cp.tile([1, B_LOC * P], fp32, tag="q_row")
            dma(q_row[:, :], d_q.ap())
            filc = cp.tile([128, 1], fp32, tag="filc")
            dma(filc[:, :], d_fil.ap().rearrange("(f o) -> f o", o=1))
            volr = cp.tile([128, V], fp32, tag="volr")
            dma(volr[:, :], bass.AP(tensor=d_vol, offset=0, ap=[[0, 128], [1, V]]))
            lf_sb = cp.tile([128, B_LOC, P], fp32, tag="lf_sb")
            dma(lf_sb[:, :, :], d_lf.ap())
            lt_sb = []
            for s in range(NSETS):
                t = cp.tile([128, F], fp32, tag=f"lt{s}", name=f"lt{s}")
                dma(t[:, :], d_lt.ap()[s * 128:(s + 1) * 128, :])
                lt_sb.append(t)

            # ---- X12 = bf16 split-precision rows for S = x^2 - 2qx + q^2 ----
            # 3-way hi/mid/lo bf16 splits keep ~fp32 accuracy at full bf16
            # matmul rate. Rows (K=12):
            #   0..2 : a_h a_m a_l  (a = x^2)        weights 1
            #   3..5 : b_h b_h b_h  (b = x)          weights w_h w_m w_l
            #   6..7 : b_m b_m                       weights w_h w_m
            #   8    : b_l                           weights w_h
            #   9..11: 1 1 1                         weights c_h c_m c_l
            # where w = -2q, c = q^2 per (b,p) column.
            x_ft = cp.tile([128, V], fp32, tag="x_ft")
            nc.vector.tensor_scalar(x_ft[:, :], volr[:, :], filc[:, 0:1], None,
                                    op0=OP.mult)
            xsq_ft = cp.tile([128, V], fp32, tag="xsq_ft")
            nc.vector.tensor_tensor(xsq_ft[:, :], x_ft[:, :], x_ft[:, :], OP.mult)
            ones_bf = cp.tile([128, V], bf16, tag="ones_bf")
            nc.vector.memset(ones_bf[:, :], 1.0)

            def split3(val32, pfx):
                """fp32 (128,V) -> three bf16 (128,V) tiles h+m+l ~= val."""
                h = cp.tile([128, V], bf16, tag=f"{pfx}h", name=f"{pfx}h")
                m = cp.tile([128, V], bf16, tag=f"{pfx}m", name=f"{pfx}m")
                l = cp.tile([128, V], bf16, tag=f"{pfx}l", name=f"{pfx}l")
                r1 = cp.tile([128, V], fp32, tag=f"{pfx}r1", name=f"{pfx}r1")
                r2 = cp.tile([128, V], fp32, tag=f"{pfx}r2", name=f"{pfx}r2")
                nc.vector.tensor_copy(h[:, :], val32[:, :])
                nc.vector.tensor_tensor(r1[:, :], val32[:, :], h[:, :], OP.subtract)
                nc.vector.tensor_copy(m[:, :], r1[:, :])
                nc.vector.tensor_tensor(r2[:, :], r1[:, :], m[:, :], OP.subtract)
                nc.vector.tensor_copy(l[:, :], r2[:, :])
                return h, m, l

            a_h, a_m, a_l = split3(xsq_ft, "a")
            b_h, b_m, b_l = split3(x_ft, "b")

            # ---- W_q per set: 12 bf16 rows (see X12 comment) ----
            Wq = []
            # scratch rows at partition 0: fp32 w, c and residuals + bf16 outs
            wt32 = cp.tile([1, 128], fp32, tag="wt32")
            ct32 = cp.tile([1, 128], fp32, tag="ct32")
            res1 = cp.tile([1, 128], fp32, tag="res1")
            res2 = cp.tile([1, 128], fp32, tag="res2")

            def split3_row(val32, srow, cols):
                """bf16-split val32 (1,128) into 128-col slices of srow."""
                cur = val32
                for i, cidx in enumerate(cols):
                    hb = srow[0:1, cidx * 128:(cidx + 1) * 128]
                    nc.vector.tensor_copy(hb, cur[:, :])
                    if i < len(cols) - 1:
                        dst = res1 if cur is not res1 else res2
                        nc.vector.tensor_tensor(dst[:, :], cur[:, :], hb,
                                                OP.subtract)
                        cur = dst

            for s in range(NSETS):
                # all 12 rows packed in one (1, 12*128) bf16 scratch, one DMA
                srow = cp.tile([1, 12 * 128], bf16, tag=f"srow{s}",
                               name=f"srow{s}")
                w = cp.tile([12, 128], bf16, tag=f"Wq{s}", name=f"Wq{s}")
                qs = q_row[0:1, s * 128:(s + 1) * 128]
                nc.vector.memset(srow[0:1, 0:3 * 128], 1.0)
                nc.vector.tensor_scalar_mul(wt32[:, :], qs, -2.0)
                nc.vector.tensor_tensor(ct32[:, :], qs, qs, OP.mult)
                split3_row(wt32, srow, [3, 4, 5])   # w_h w_m w_l
                nc.vector.tensor_copy(srow[0:1, 6 * 128:7 * 128],
                                      srow[0:1, 3 * 128:4 * 128])  # w_h
                nc.vector.tensor_copy(srow[0:1, 7 * 128:8 * 128],
                                      srow[0:1, 4 * 128:5 * 128])  # w_m
                nc.vector.tensor_copy(srow[0:1, 8 * 128:9 * 128],
                                      srow[0:1, 3 * 128:4 * 128])  # w_h
                split3_row(ct32, srow, [9, 10, 11])  # c_h c_m c_l
                dma(w[:, :], srow[0:1, :])
                Wq.append(w)

            # X row-writes hit a single SBUF partition each (~2.8 GB/s), so
            # split into column pieces: PE-1 starts after piece 0 while the
            # rest streams in (range-granular Tile deps).
            X = cp.tile([12, NVF], bf16, tag="X")
            NPIECE = 4
            FPP = 128 // NPIECE                      # f's per piece
            srcs = [a_h, a_m, a_l, b_h, b_h, b_h, b_m, b_m,
                    b_l, ones_bf, ones_bf, ones_bf]
            for p in range(NPIECE):
                for r, t in enumerate(srcs):
                    dma3(X[r:r + 1, p * FPP * V:(p + 1) * FPP * V],
                         t[p * FPP:(p + 1) * FPP, :])

            # ---- softmax denominators: Z[f, b] = sum_p exp(logits) ----
            lf_sb = cp.tile([128, B_LOC, P], fp32, tag="lf_sb")
            dma(lf_sb[:, :, :], d_lf.ap())
            el_f = cp.tile([128, B_LOC, P], fp32, tag="el_f")
            nc.scalar.activation(el_f[:, :, :], lf_sb[:, :, :], AF.Exp)
            Z = cp.tile([128, B_LOC], fp32, tag="Z")
            nc.vector.tensor_reduce(Z[:, :], el_f[:, :, :], mybir.AxisListType.X,
                                    OP.add)
            Zinv = cp.tile([128, B_LOC], fp32, tag="Zinv")
            nc.vector.reciprocal(Zinv[:, :], Z[:, :])
            # bounce transposed to [b][f] so reloads are contiguous rows
            dma(bass.AP(tensor=d_zb, offset=0, ap=[[1, 128], [128, B_LOC]]),
                Zinv[:, :])

            # ---- W_amp per set: block-diag fp16 softmax weights ----
            # W_amp[k=(b,p), 2f+h] = amp[b,f,p] for k//64==h else 0
            Wamp = []
            for s in range(NSETS):
                elt = cp.tile([128, F], fp32, tag=f"elt{s}", name=f"elt{s}")
                nc.scalar.activation(elt[:, :], lt_sb[s][:, :], AF.Exp)
                zr = cp.tile([128, F], fp32, tag=f"zr{s}", name=f"zr{s}")
                for h in range(2):
                    dma(zr[h * 64:(h + 1) * 64, :],
                        bass.AP(tensor=d_zb, offset=(2 * s + h) * F,
                                ap=[[0, 64], [1, F]]))
                w = cp.tile([128, 2 * F], fp16, tag=f"Wamp{s}", name=f"Wamp{s}")
                nc.vector.memset(w[:, :], 0.0)
                for h in range(2):
                    nc.vector.tensor_tensor(
                        w[h * 64:(h + 1) * 64, h:2 * F:2],
                        elt[h * 64:(h + 1) * 64, :],
                        zr[h * 64:(h + 1) * 64, :],
                        OP.mult,
                    )
                Wamp.append(w)

            # ---- main pipeline ----
            # psum out accumulator reused across sets, cols [vh*256+2*f+b']
            sO = psO.tile([128, 2 * 2 * F], fp32, tag="O", name="sO")
            for s in range(NSETS):
                for (g0, gc) in GROUPS:
                    sS = psS.tile([128, GROUP], fp32, tag="S", name="sS")
                    for ci in range(gc // 512):
                        off = g0 + ci * 512
                        nc.tensor.matmul(
                            sS[:, ci * 512:(ci + 1) * 512],
                            Wq[s][:, :],
                            X[:, off:off + 512],
                            start=True, stop=True,
                        )
                    E = ep.tile([128, GROUP], fp16, tag="E", name="E")
                    nc.scalar.activation(E[:, 0:gc], sS[:, 0:gc], AF.Exp,
                                         scale=float(minus_c))
                    for fr in range(gc // V):
                        f = g0 // V + fr                       # global f
                        for vh in range(2):
                            base = vh * 2 * F
                            nc.tensor.matmul(
                                sO[:, base + 2 * f:base + 2 * f + 2],
                                E[:, fr * V + vh * 128:fr * V + vh * 128 + 128],
                                Wamp[s][:, 2 * f:2 * f + 2],
                                start=True, stop=True,
                            )
                # drain psum -> sbuf -> DRAM out[s, v, f, b'] (contiguous)
                for vh in range(2):
                    ob = cp.tile([128, 2 * F], fp32, tag=f"ob{vh}",
                                 name=f"ob{vh}")
                    base = vh * 2 * F
                    nc.vector.tensor_copy(ob[:, :], sO[:, base:base + 2 * F])
                    nc.sync.dma_start(
                        d_out.ap()[s:s + 1, vh * 128:(vh + 1) * 128, :, :],
                        ob[:, :],
                    )

    nc.compile()
    return nc


def _get_nc(minus_c):
    key = float(minus_c)
    if key not in _cache:
        _cache[key] = _build(key)
    return _cache[key]


def kernel(q2_obs_scaled, amplitude_logits, volumes, filters, sigma,
           _trace=False, _tmpdir=None):
    from concourse.bass_utils import run_bass_kernel_spmd

    minus_c = -0.5 / (float(np.asarray(sigma).reshape(())) + 0.001) ** 2
    nc = _get_nc(minus_c)

    q = np.ascontiguousarray(np.asarray(q2_obs_scaled, dtype=np.float32))
    lg = np.asarray(amplitude_logits, dtype=np.float32).reshape(B, F, P)
    vol = np.ascontiguousarray(np.asarray(volumes, dtype=np.float32).reshape(V))
    fil = np.ascontiguousarray(np.asarray(filters, dtype=np.float32).reshape(F))
    sig = np.asarray(sigma, dtype=np.float32).reshape(1)

    in_maps = []
    for i in range(NCORES):
        bsl = slice(i * B_LOC, (i + 1) * B_LOC)
        lgc = lg[bsl]                                    # (B_LOC, F, P)
        in_maps.append({
            "q": np.ascontiguousarray(q[bsl].reshape(B_LOC * P)),
            "lf": np.ascontiguousarray(lgc.transpose(1, 0, 2)),   # (F,B_LOC,P)
            "lt": np.ascontiguousarray(
                lgc.transpose(0, 2, 1).reshape(B_LOC * P, F)),    # ((b,p),F)
            "vol": vol,
            "fil": fil,
            "sig": sig,
        })

    kw = {}
    if _trace:
        kw = {"trace": True, "tmpdir": _tmpdir}
    res = run_bass_kernel_spmd(nc, in_maps, core_ids=list(range(NCORES)), **kw)

    out = np.empty((B, V, F), dtype=np.float32)
    for i in range(NCORES):
        oc = res.results[i]["out"]                       # (NSETS, V, F, 2)
        for s in range(NSETS):
            for h in range(2):
                out[i * B_LOC + 2 * s + h] = oc[s, :, :, h]
    if _trace:
        return out, res
    return out
